# revision 14
# baseline (speedup 1.0000x reference)
"""Trainium2 Bass kernel for nn_CuteInferLinear (quantized linear, fp8-e4m3fn emulation).

Math (per reference):
  xq, xs = quantize(x, chunk=128)   per-row/per-128-col-group fp8_e4m3fn quant
  wq, ws = quantize(W, chunk=128)
  out = (xq*rep(xs)) @ (wq*rep(ws)).T + bias        -> bf16

Implementation notes:
  * TRN2's float8e4 is e4m3 with max +-240, NOT OCP e4m3fn (max 448). Quantizing
    with scale amax/224 instead of amax/448 (and dequantizing to match) is
    bit-equivalent for normals (pure exponent shift), so we use 224.
  * Dequantized xd/wd are rounded to bf16 for the TensorEngine matmul (PSUM f32
    accumulate). End-to-end rel-l2 error vs the f32 reference ~3.6e-3.
  * Tensor-parallel over 8 cores: W/bias/out sharded on N, x replicated.
  * Preproc: broadcast-AP (0-stride) tensor_tensor ops -- quant on DVE,
    dequant on GPSIMD -- one instruction per K-chunk.
  * DMA_TRANSPOSE issue cost on the sequencer is ~1.25us per instruction
    regardless of size, so transposes are batched big: xd goes x->xd_dram
    (natural) -> [1024,128] xbar transposes -> xdT_dram (K-major), and matmul
    panels load with ONE plain DMA each. wd transposes straight into the
    SBUF-resident wdT.
  * bias is added via a K=1 matmul row that opens each PSUM accumulation
    group (start=True), so eviction is a pure ACT copy (DVE stays free).
  * All HWDGE DMA issue stays on nc.sync: splitting across the SP+ACT rings
    corrupts xbar transposes at scale (verified empirically).
"""

import numpy as np
import ml_dtypes

import concourse.bass as bass
import concourse.mybir as mybir
import concourse.tile as tile
from concourse import bacc

P = 128
FP8_SCALE = 224.0
EPS = 1e-4

BF16 = mybir.dt.bfloat16
F32 = mybir.dt.float32
FP8 = mybir.dt.float8e4


def _bcast(stat_ap, g0, ng, c):
    """View stat[:, g0:g0+ng] as [P, ng, c] with 0-stride inner dim."""
    base = stat_ap[:, g0 : g0 + ng]
    return bass.AP(tensor=base.tensor, offset=base.offset, ap=[base.ap[0], base.ap[1], [0, c]])


def build_core_program(
    M: int,
    K: int,
    NL: int,
    MP: int = 256,       # m-panel rows per xdT SBUF load
    NBLK: int = 512,     # psum block (free dim per matmul)
    PREK: int = 2048,    # preproc K chunk
    XBLKS=None,          # x transpose block row counts (graduated)
    NSPL: int = 4,       # N splits of the matmul loop / W gating
    num_devices: int = 8,
):
    KO = K // P
    assert K % P == 0 and M % MP == 0 and MP % P == 0
    NBLK = min(NBLK, NL)
    assert NL % NBLK == 0
    NB = NL // NBLK
    MPT = MP // P
    PREK = min(PREK, K)
    assert K % PREK == 0 and PREK % P == 0
    PCH = K // PREK
    GC = PREK // P
    assert NL % NSPL == 0 and NB % NSPL == 0
    NW = NL // NSPL          # rows per W gating split
    NBH = NB // NSPL         # psum blocks per half
    if XBLKS is None:
        XBLKS = [256, 512, 1280, 2048] if M == 4096 else [M]
    assert sum(XBLKS) == M and all(b % MP == 0 for b in XBLKS)

    nc = bacc.Bacc(
        "TRN2",
        target_bir_lowering=False,
        debug=False,
        enable_asserts=True,
        num_devices=num_devices,
    )

    x_in = nc.dram_tensor("x", [M, K], BF16, kind="ExternalInput").ap()
    w_in = nc.dram_tensor("w", [NL, K], BF16, kind="ExternalInput").ap()
    b_in = nc.dram_tensor("bias", [NL], BF16, kind="ExternalInput").ap()
    out = nc.dram_tensor("out", [M, NL], BF16, kind="ExternalOutput").ap()
    xd_dram = nc.dram_tensor("xd_scratch", [M, K], BF16).ap()
    wd_dram = nc.dram_tensor("wd_scratch", [NL, K], BF16).ap()
    xdt_dram = nc.dram_tensor("xdt_scratch", [KO, P, M], BF16).ap()

    with tile.TileContext(nc) as tc:
        with (
            tc.tile_pool(name="const", bufs=1) as const,
            tc.tile_pool(name="wdt", bufs=1) as wdt_pool,
            tc.tile_pool(name="xdt", bufs=2) as xdt_pool,
            tc.tile_pool(name="stg", bufs=2) as stg_pool,
            tc.tile_pool(name="prepx", bufs=PCH + 1) as prepx,
            tc.tile_pool(name="prepqd", bufs=2) as prepqd,
            tc.tile_pool(name="stat", bufs=3) as stat,
            tc.tile_pool(name="psum", bufs=6, space="PSUM") as psum_pool,
            tc.tile_pool(name="outp", bufs=2) as outp,
        ):
            # bias row (partition 0) + ones column for the K=1 bias matmul
            bias_sb = const.tile([P, NL], BF16)
            nc.sync.dma_start(out=bias_sb[0:1, :], in_=b_in[None, :])
            ones_sb = const.tile([P, NBLK], BF16)
            nc.vector.memset(ones_sb[:], 1.0)

            def quant_dequant_rows(src, dst, row0):
                """fp8 quantize+dequantize one [P, K] row-tile src->dst (DRAM)."""
                xts = []
                amax = stat.tile([P, KO], F32, tag="amax")
                for c in range(PCH):
                    xt = prepx.tile([P, PREK], BF16, tag="pt_in")
                    nc.sync.dma_start(out=xt[:], in_=src[row0 : row0 + P, bass.ts(c, PREK)])
                    nc.vector.tensor_reduce(
                        out=amax[:, c * GC : (c + 1) * GC],
                        in_=xt.rearrange("p (g c) -> p g c", c=P),
                        axis=mybir.AxisListType.X,
                        op=mybir.AluOpType.max,
                        apply_absolute_value=True,
                    )
                    xts.append(xt)
                nc.vector.tensor_scalar_max(amax[:], amax[:], EPS)
                inv = stat.tile([P, KO], F32, tag="inv")
                nc.vector.reciprocal(out=inv[:], in_=amax[:])
                nc.vector.tensor_scalar_mul(inv[:], inv[:], FP8_SCALE)
                sc = stat.tile([P, KO], F32, tag="sc")
                nc.vector.tensor_scalar_mul(sc[:], amax[:], 1.0 / FP8_SCALE)
                for c in range(PCH):
                    xt = xts[c]
                    qt = prepqd.tile([P, PREK], FP8, tag="pt_q")
                    dt_ = prepqd.tile([P, PREK], BF16, tag="pt_d")
                    nc.vector.tensor_tensor(
                        out=qt.rearrange("p (g c) -> p g c", c=P),
                        in0=xt.rearrange("p (g c) -> p g c", c=P),
                        in1=_bcast(inv, c * GC, GC, P),
                        op=mybir.AluOpType.mult,
                    )
                    nc.vector.tensor_tensor(
                        out=dt_.rearrange("p (g c) -> p g c", c=P),
                        in0=qt.rearrange("p (g c) -> p g c", c=P),
                        in1=_bcast(sc, c * GC, GC, P),
                        op=mybir.AluOpType.mult,
                    )
                    nc.sync.dma_start(out=dst[row0 : row0 + P, bass.ts(c, PREK)], in_=dt_[:])

            def x_block_transpose(m0, rows):
                """xd_dram rows [m0, m0+rows) -> xdT_dram [ko, p, mrange]."""
                for ko in range(KO):
                    stg = stg_pool.tile([P, max(XBLKS)], BF16, tag="stg")
                    nc.sync.dma_start_transpose(
                        out=stg[:, :rows],
                        in_=xd_dram[m0 : m0 + rows, bass.ts(ko, P)],
                    )
                    nc.sync.dma_start(out=xdt_dram[ko, :, m0 : m0 + rows], in_=stg[:, :rows])

            wdT = wdt_pool.tile([P, KO, NL], BF16)

            def wd_transpose(ws):
                for ko in range(KO):
                    nc.sync.dma_start_transpose(
                        out=wdT[:, ko, bass.ds(ws * NW, NW)],
                        in_=wd_dram[bass.ds(ws * NW, NW), bass.ts(ko, P)],
                    )

            # ---- head: x block 0 + W half 0, their transposes ----
            for t in range(XBLKS[0] // P):
                quant_dequant_rows(x_in, xd_dram, t * P)
            for t in range(NW // P):
                quant_dequant_rows(w_in, wd_dram, t * P)
            wd_transpose(0)
            x_block_transpose(0, XBLKS[0])

            # PE warmer: dummy matmuls keep the PE busy (and HAM un-throttled)
            # while the head preproc runs; results are discarded.
            warm_sink = nc.dram_tensor("warm_sink", [1, NBLK], F32).ap()
            dpsum = psum_pool.tile([P, NBLK], F32, tag="ps", name="warmps")
            N_WARM = 500
            for i in range(N_WARM):
                nc.tensor.matmul(
                    dpsum[:], ones_sb[:, :P], ones_sb[:],
                    start=(i == 0), stop=(i == N_WARM - 1),
                )
            wsink = const.tile([1, NBLK], F32)
            nc.scalar.copy(out=wsink[:], in_=dpsum[0:1, :])
            nc.sync.dma_start(out=warm_sink, in_=wsink[:])

            # ---- all deferred preproc/transposes, emitted up front; the panel
            # loads live on the other HWDGE ring so they don't queue behind this
            m0 = XBLKS[0]
            for rows in XBLKS[1:]:
                for t in range(rows // P):
                    quant_dequant_rows(x_in, xd_dram, m0 + t * P)
                x_block_transpose(m0, rows)
                m0 += rows
            for ws in range(1, NSPL):
                for t in range(NW // P):
                    quant_dequant_rows(w_in, wd_dram, ws * NW + t * P)
                wd_transpose(ws)

            n_panels = M // MP

            for half in range(NSPL):
                nh0 = half * NW
                for mp in range(n_panels):
                    mrow0 = mp * MP
                    xdT = xdt_pool.tile([P, KO, MP], BF16, tag="xdT")
                    nc.scalar.dma_start(
                        out=xdT[:],
                        in_=xdt_dram.rearrange("ko p m -> p ko m")[:, :, mrow0 : mrow0 + MP],
                    )
                    for ms in range(MPT):
                        ot = outp.tile([P, NW], BF16, tag="osb")
                        psums = [
                            psum_pool.tile([P, NBLK], F32, tag="ps", name=f"ps{i}")
                            for i in range(NBH)
                        ]
                        for nbi in range(NBH):
                            # bias row opens the accumulation group (K=1 matmul)
                            nc.tensor.matmul(
                                psums[nbi][:],
                                ones_sb[0:1, :P],
                                bias_sb[0:1, bass.ds(nh0 + nbi * NBLK, NBLK)],
                                start=True,
                                stop=False,
                            )
                        for ko in range(KO):
                            for nbi in range(NBH):
                                nc.tensor.matmul(
                                    psums[nbi][:],
                                    xdT[:, ko, bass.ts(ms, P)],
                                    wdT[:, ko, bass.ds(nh0 + nbi * NBLK, NBLK)],
                                    start=False,
                                    stop=(ko == KO - 1),
                                )
                        for nbi in range(NBH):
                            nc.scalar.copy(out=ot[:, bass.ts(nbi, NBLK)], in_=psums[nbi][:])
                        nc.scalar.dma_start(
                            out=out[
                                mrow0 + ms * P : mrow0 + (ms + 1) * P,
                                bass.ds(nh0, NW),
                            ],
                            in_=ot[:],
                        )


    nc.compile()
    return nc


_CACHE = {}


def _get_program(M, K, NL, **kw):
    key = (M, K, NL, tuple(sorted(kw.items())))
    if key not in _CACHE:
        _CACHE[key] = build_core_program(M, K, NL, **kw)
    return _CACHE[key]


def kernel(x, W, bias, chunk_size=128, int8=0, **_unused):
    """Full-input entry: shards across 8 NeuronCores (column-parallel) and
    returns the full [M, N] output."""
    from concourse.bass_utils import run_bass_kernel_spmd

    assert int(chunk_size) == 128 and int(int8) == 0
    x = np.asarray(x)
    W = np.asarray(W)
    bias = np.asarray(bias)
    M, K = x.shape
    N = W.shape[0]
    n_cores = 8
    assert N % n_cores == 0
    NL = N // n_cores

    nc = _get_program(M, K, NL)

    bf = ml_dtypes.bfloat16
    xb = np.ascontiguousarray(x.astype(bf, copy=False))
    in_maps = []
    for i in range(n_cores):
        in_maps.append(
            {
                "x": xb,
                "w": np.ascontiguousarray(W[i * NL : (i + 1) * NL].astype(bf, copy=False)),
                "bias": np.ascontiguousarray(bias[i * NL : (i + 1) * NL].astype(bf, copy=False)),
            }
        )

    res = run_bass_kernel_spmd(nc, in_maps, core_ids=list(range(n_cores)))
    outs = [res.results[i]["out"] for i in range(n_cores)]
    full = np.concatenate(outs, axis=1)
    return full.astype(x.dtype, copy=False)


# revision 16
# speedup vs baseline: 1.1002x; 1.1002x over previous
"""Trainium2 Bass kernel for nn_CuteInferLinear (quantized linear, fp8-e4m3fn emulation).

Math (per reference):
  xq, xs = quantize(x, chunk=128)   per-row/per-128-col-group fp8_e4m3fn quant
  wq, ws = quantize(W, chunk=128)
  out = (xq*rep(xs)) @ (wq*rep(ws)).T + bias        -> bf16

Implementation notes:
  * TRN2's float8e4 is e4m3 with max +-240, NOT OCP e4m3fn (max 448). Quantizing
    with scale amax/224 instead of amax/448 (and dequantizing to match) is
    bit-equivalent for normals (pure exponent shift), so we use 224.
  * Dequantized xd/wd are rounded to bf16 for the TensorEngine matmul (PSUM f32
    accumulate). End-to-end rel-l2 error vs the f32 reference ~3.6e-3.
  * Tensor-parallel over 8 cores: W/bias/out sharded on N, x replicated.
  * Preproc: broadcast-AP (0-stride) tensor_tensor ops -- quant on DVE,
    dequant on GPSIMD -- one instruction per K-chunk.
  * DMA_TRANSPOSE issue cost on the sequencer is ~1.25us per instruction
    regardless of size, so transposes are batched big: xd goes x->xd_dram
    (natural) -> [1024,128] xbar transposes -> xdT_dram (K-major), and matmul
    panels load with ONE plain DMA each. wd transposes straight into the
    SBUF-resident wdT.
  * bias is added via a K=1 matmul row that opens each PSUM accumulation
    group (start=True), so eviction is a pure ACT copy (DVE stays free).
  * All HWDGE DMA issue stays on nc.sync: splitting across the SP+ACT rings
    corrupts xbar transposes at scale (verified empirically).
"""

import numpy as np
import ml_dtypes

import concourse.bass as bass
import concourse.mybir as mybir
import concourse.tile as tile
from concourse import bacc

P = 128
FP8_SCALE = 224.0
EPS = 1e-4

BF16 = mybir.dt.bfloat16
F32 = mybir.dt.float32
FP8 = mybir.dt.float8e4


def _bcast(stat_ap, g0, ng, c):
    """View stat[:, g0:g0+ng] as [P, ng, c] with 0-stride inner dim."""
    base = stat_ap[:, g0 : g0 + ng]
    return bass.AP(tensor=base.tensor, offset=base.offset, ap=[base.ap[0], base.ap[1], [0, c]])


def build_core_program(
    M: int,
    K: int,
    NL: int,
    MP: int = 256,       # m-panel rows per xdT SBUF load
    NBLK: int = 512,     # psum block (free dim per matmul)
    PREK: int = 2048,    # preproc K chunk
    XBLKS=None,          # x transpose block row counts (graduated)
    NSPL: int = 1,       # N splits of the matmul loop
    WSPL: int = 4,       # W transpose gating quarters
    num_devices: int = 8,
):
    KO = K // P
    assert K % P == 0 and M % MP == 0 and MP % P == 0
    NBLK = min(NBLK, NL)
    assert NL % NBLK == 0
    NB = NL // NBLK
    MPT = MP // P
    PREK = min(PREK, K)
    assert K % PREK == 0 and PREK % P == 0
    PCH = K // PREK
    GC = PREK // P
    assert NL % NSPL == 0 and NB % NSPL == 0 and NL % WSPL == 0
    NW = NL // NSPL          # matmul pass width
    NBH = NB // NSPL         # psum blocks per pass
    WGR = NL // WSPL         # rows per W transpose gate
    if XBLKS is None:
        XBLKS = [256, 512, 1280, 2048] if M == 4096 else [M]
    assert sum(XBLKS) == M and all(b % MP == 0 for b in XBLKS)

    nc = bacc.Bacc(
        "TRN2",
        target_bir_lowering=False,
        debug=False,
        enable_asserts=True,
        num_devices=num_devices,
    )

    x_in = nc.dram_tensor("x", [M, K], BF16, kind="ExternalInput").ap()
    w_in = nc.dram_tensor("w", [NL, K], BF16, kind="ExternalInput").ap()
    b_in = nc.dram_tensor("bias", [NL], BF16, kind="ExternalInput").ap()
    out = nc.dram_tensor("out", [M, NL], BF16, kind="ExternalOutput").ap()
    xd_dram = nc.dram_tensor("xd_scratch", [M, K], BF16).ap()
    wd_dram = nc.dram_tensor("wd_scratch", [NL, K], BF16).ap()
    xdt_dram = nc.dram_tensor("xdt_scratch", [KO, P, M], BF16).ap()

    with tile.TileContext(nc) as tc:
        with (
            tc.tile_pool(name="const", bufs=1) as const,
            tc.tile_pool(name="wdt", bufs=1) as wdt_pool,
            tc.tile_pool(name="xdt", bufs=2) as xdt_pool,
            tc.tile_pool(name="stg", bufs=1) as stg_pool,
            tc.tile_pool(name="prepx", bufs=PCH + 1) as prepx,
            tc.tile_pool(name="prepqd", bufs=2) as prepqd,
            tc.tile_pool(name="stat", bufs=3) as stat,
            tc.tile_pool(name="psum", bufs=6, space="PSUM") as psum_pool,
            tc.tile_pool(name="outp", bufs=2) as outp,
        ):
            # bias row (partition 0) + ones column for the K=1 bias matmul
            bias_sb = const.tile([P, NL], BF16)
            nc.sync.dma_start(out=bias_sb[0:1, :], in_=b_in[None, :])
            ones_sb = const.tile([P, NBLK], BF16)
            nc.vector.memset(ones_sb[:], 1.0)

            def quant_dequant_rows(src, dst, row0):
                """fp8 quantize+dequantize one [P, K] row-tile src->dst (DRAM)."""
                xts = []
                amax = stat.tile([P, KO], F32, tag="amax")
                for c in range(PCH):
                    xt = prepx.tile([P, PREK], BF16, tag="pt_in")
                    nc.sync.dma_start(out=xt[:], in_=src[row0 : row0 + P, bass.ts(c, PREK)])
                    nc.vector.tensor_reduce(
                        out=amax[:, c * GC : (c + 1) * GC],
                        in_=xt.rearrange("p (g c) -> p g c", c=P),
                        axis=mybir.AxisListType.X,
                        op=mybir.AluOpType.max,
                        apply_absolute_value=True,
                    )
                    xts.append(xt)
                nc.vector.tensor_scalar_max(amax[:], amax[:], EPS)
                inv = stat.tile([P, KO], F32, tag="inv")
                nc.vector.reciprocal(out=inv[:], in_=amax[:])
                nc.vector.tensor_scalar_mul(inv[:], inv[:], FP8_SCALE)
                sc = stat.tile([P, KO], F32, tag="sc")
                nc.vector.tensor_scalar_mul(sc[:], amax[:], 1.0 / FP8_SCALE)
                for c in range(PCH):
                    xt = xts[c]
                    qt = prepqd.tile([P, PREK], FP8, tag="pt_q")
                    dt_ = prepqd.tile([P, PREK], BF16, tag="pt_d")
                    nc.vector.tensor_tensor(
                        out=qt.rearrange("p (g c) -> p g c", c=P),
                        in0=xt.rearrange("p (g c) -> p g c", c=P),
                        in1=_bcast(inv, c * GC, GC, P),
                        op=mybir.AluOpType.mult,
                    )
                    nc.vector.tensor_tensor(
                        out=dt_.rearrange("p (g c) -> p g c", c=P),
                        in0=qt.rearrange("p (g c) -> p g c", c=P),
                        in1=_bcast(sc, c * GC, GC, P),
                        op=mybir.AluOpType.mult,
                    )
                    nc.sync.dma_start(out=dst[row0 : row0 + P, bass.ts(c, PREK)], in_=dt_[:])

            def x_block_transpose(m0, rows):
                """xd_dram rows [m0, m0+rows) -> xdT_dram [ko, p, mrange]."""
                for ko in range(KO):
                    stg = stg_pool.tile([P, max(XBLKS)], BF16, tag="stg")
                    nc.sync.dma_start_transpose(
                        out=stg[:, :rows],
                        in_=xd_dram[m0 : m0 + rows, bass.ts(ko, P)],
                    )
                    nc.sync.dma_start(out=xdt_dram[ko, :, m0 : m0 + rows], in_=stg[:, :rows])

            wdT = wdt_pool.tile([P, KO, NL], BF16)

            def wd_transpose(ws):
                for ko in range(KO):
                    nc.sync.dma_start_transpose(
                        out=wdT[:, ko, bass.ds(ws * WGR, WGR)],
                        in_=wd_dram[bass.ds(ws * WGR, WGR), bass.ts(ko, P)],
                    )

            # ---- head: x block 0, then W in gated quarters ----
            for t in range(XBLKS[0] // P):
                quant_dequant_rows(x_in, xd_dram, t * P)
            for ws in range(WSPL):
                for t in range(WGR // P):
                    quant_dequant_rows(w_in, wd_dram, ws * WGR + t * P)
                wd_transpose(ws)
            x_block_transpose(0, XBLKS[0])

            # PE warmer: dummy matmuls keep the PE busy (and HAM un-throttled)
            # while the head preproc runs; results are discarded.
            warm_sink = nc.dram_tensor("warm_sink", [1, NBLK], F32).ap()
            dpsum = psum_pool.tile([P, NBLK], F32, tag="ps", name="warmps")
            N_WARM = 500
            for i in range(N_WARM):
                nc.tensor.matmul(
                    dpsum[:], ones_sb[:, :P], ones_sb[:],
                    start=(i == 0), stop=(i == N_WARM - 1),
                )
            wsink = const.tile([1, NBLK], F32)
            nc.scalar.copy(out=wsink[:], in_=dpsum[0:1, :])
            nc.sync.dma_start(out=warm_sink, in_=wsink[:])

            # x blocks 1+ get JIT-emitted inside the panel loop, right after a
            # panel load; panel loads are on the other HWDGE ring so they don't
            # queue behind this work on the sync sequencer.
            blk_starts = {}
            m0 = XBLKS[0]
            for bi, rows in enumerate(XBLKS[1:]):
                blk_starts[(m0 - XBLKS[bi]) // MP] = (m0, rows)
                m0 += rows

            n_panels = M // MP

            for half in range(NSPL):
                nh0 = half * NW
                for mp in range(n_panels):
                    mrow0 = mp * MP
                    xdT = xdt_pool.tile([P, KO, MP], BF16, tag="xdT")
                    nc.scalar.dma_start(
                        out=xdT[:],
                        in_=xdt_dram.rearrange("ko p m -> p ko m")[:, :, mrow0 : mrow0 + MP],
                    )
                    if half == 0 and mp in blk_starts:
                        bm0, brows = blk_starts[mp]
                        for t in range(brows // P):
                            quant_dequant_rows(x_in, xd_dram, bm0 + t * P)
                        x_block_transpose(bm0, brows)
                    for ms in range(MPT):
                        ot = outp.tile([P, NW], BF16, tag="osb")
                        psums = [
                            psum_pool.tile([P, NBLK], F32, tag="ps", name=f"ps{i}")
                            for i in range(NBH)
                        ]
                        for nbi in range(NBH):
                            # bias row opens the accumulation group (K=1 matmul)
                            nc.tensor.matmul(
                                psums[nbi][:],
                                ones_sb[0:1, :P],
                                bias_sb[0:1, bass.ds(nh0 + nbi * NBLK, NBLK)],
                                start=True,
                                stop=False,
                            )
                        for ko in range(KO):
                            for nbi in range(NBH):
                                nc.tensor.matmul(
                                    psums[nbi][:],
                                    xdT[:, ko, bass.ts(ms, P)],
                                    wdT[:, ko, bass.ds(nh0 + nbi * NBLK, NBLK)],
                                    start=False,
                                    stop=(ko == KO - 1),
                                )
                        for nbi in range(NBH):
                            nc.scalar.copy(out=ot[:, bass.ts(nbi, NBLK)], in_=psums[nbi][:])
                        nc.scalar.dma_start(
                            out=out[
                                mrow0 + ms * P : mrow0 + (ms + 1) * P,
                                bass.ds(nh0, NW),
                            ],
                            in_=ot[:],
                        )


    nc.compile()
    return nc


_CACHE = {}


def _get_program(M, K, NL, **kw):
    key = (M, K, NL, tuple(sorted(kw.items())))
    if key not in _CACHE:
        _CACHE[key] = build_core_program(M, K, NL, **kw)
    return _CACHE[key]


def kernel(x, W, bias, chunk_size=128, int8=0, **_unused):
    """Full-input entry: shards across 8 NeuronCores (column-parallel) and
    returns the full [M, N] output."""
    from concourse.bass_utils import run_bass_kernel_spmd

    assert int(chunk_size) == 128 and int(int8) == 0
    x = np.asarray(x)
    W = np.asarray(W)
    bias = np.asarray(bias)
    M, K = x.shape
    N = W.shape[0]
    n_cores = 8
    assert N % n_cores == 0
    NL = N // n_cores

    nc = _get_program(M, K, NL)

    bf = ml_dtypes.bfloat16
    xb = np.ascontiguousarray(x.astype(bf, copy=False))
    in_maps = []
    for i in range(n_cores):
        in_maps.append(
            {
                "x": xb,
                "w": np.ascontiguousarray(W[i * NL : (i + 1) * NL].astype(bf, copy=False)),
                "bias": np.ascontiguousarray(bias[i * NL : (i + 1) * NL].astype(bf, copy=False)),
            }
        )

    res = run_bass_kernel_spmd(nc, in_maps, core_ids=list(range(n_cores)))
    outs = [res.results[i]["out"] for i in range(n_cores)]
    full = np.concatenate(outs, axis=1)
    return full.astype(x.dtype, copy=False)


# revision 17
# speedup vs baseline: 1.2120x; 1.1016x over previous
"""Trainium2 Bass kernel for nn_CuteInferLinear (quantized linear, fp8-e4m3fn emulation).

Math (per reference):
  xq, xs = quantize(x, chunk=128)   per-row/per-128-col-group fp8_e4m3fn quant
  wq, ws = quantize(W, chunk=128)
  out = (xq*rep(xs)) @ (wq*rep(ws)).T + bias        -> bf16

Implementation notes:
  * TRN2's float8e4 is e4m3 with max +-240, NOT OCP e4m3fn (max 448). Quantizing
    with scale amax/224 instead of amax/448 (and dequantizing to match) is
    bit-equivalent for normals (pure exponent shift), so we use 224.
  * Dequantized xd/wd are rounded to bf16 for the TensorEngine matmul (PSUM f32
    accumulate). End-to-end rel-l2 error vs the f32 reference ~3.6e-3.
  * Tensor-parallel over 8 cores: W/bias/out sharded on N, x replicated.
  * Preproc: broadcast-AP (0-stride) tensor_tensor ops -- quant on DVE,
    dequant on GPSIMD -- one instruction per K-chunk.
  * DMA_TRANSPOSE issue cost on the sequencer is ~1.25us per instruction
    regardless of size, so transposes are batched big: xd goes x->xd_dram
    (natural) -> [1024,128] xbar transposes -> xdT_dram (K-major), and matmul
    panels load with ONE plain DMA each. wd transposes straight into the
    SBUF-resident wdT.
  * bias is added via a K=1 matmul row that opens each PSUM accumulation
    group (start=True), so eviction is a pure ACT copy (DVE stays free).
  * A run of dummy "warmer" matmuls fills the PE during the preproc head so
    the HAM clock-gate stays at full rate when the real stream begins.
  * All HWDGE DMA issue stays on nc.sync: splitting transposes across the
    SP+ACT rings corrupts xbar data at scale (verified empirically).
"""

import numpy as np
import ml_dtypes

import concourse.bass as bass
import concourse.mybir as mybir
import concourse.tile as tile
from concourse import bacc

P = 128
FP8_SCALE = 224.0
EPS = 1e-4

BF16 = mybir.dt.bfloat16
F32 = mybir.dt.float32
FP8 = mybir.dt.float8e4


def _bcast(stat_ap, g0, ng, c):
    """View stat[:, g0:g0+ng] as [P, ng, c] with 0-stride inner dim."""
    base = stat_ap[:, g0 : g0 + ng]
    return bass.AP(tensor=base.tensor, offset=base.offset, ap=[base.ap[0], base.ap[1], [0, c]])


def build_core_program(
    M: int,
    K: int,
    NL: int,
    MP: int = 256,       # m-panel rows per xdT SBUF load
    NBLK: int = 512,     # psum block (free dim per matmul)
    PREK: int = 2048,    # preproc K chunk
    XBLK: int = 1024,    # x transpose block rows
    WSPL: int = 2,       # W transpose gating splits
    N_WARM: int = 500,   # PE warmer matmuls
    num_devices: int = 8,
):
    KO = K // P
    assert K % P == 0 and M % MP == 0 and MP % P == 0
    XBLK = min(XBLK, M)
    assert M % XBLK == 0 and XBLK % MP == 0
    NBLK = min(NBLK, NL)
    assert NL % NBLK == 0
    NB = NL // NBLK
    MPT = MP // P
    PREK = min(PREK, K)
    assert K % PREK == 0 and PREK % P == 0
    PCH = K // PREK
    GC = PREK // P
    assert NL % WSPL == 0
    NW = NL // WSPL

    nc = bacc.Bacc(
        "TRN2",
        target_bir_lowering=False,
        debug=False,
        enable_asserts=True,
        num_devices=num_devices,
    )

    x_in = nc.dram_tensor("x", [M, K], BF16, kind="ExternalInput").ap()
    w_in = nc.dram_tensor("w", [NL, K], BF16, kind="ExternalInput").ap()
    b_in = nc.dram_tensor("bias", [NL], BF16, kind="ExternalInput").ap()
    out = nc.dram_tensor("out", [M, NL], BF16, kind="ExternalOutput").ap()
    xd_dram = nc.dram_tensor("xd_scratch", [M, K], BF16).ap()
    wd_dram = nc.dram_tensor("wd_scratch", [NL, K], BF16).ap()
    xdt_dram = nc.dram_tensor("xdt_scratch", [KO, P, M], BF16).ap()
    warm_sink = nc.dram_tensor("warm_sink", [1, NBLK], F32).ap()

    with tile.TileContext(nc) as tc:
        with (
            tc.tile_pool(name="const", bufs=1) as const,
            tc.tile_pool(name="wdt", bufs=1) as wdt_pool,
            tc.tile_pool(name="xdt", bufs=2) as xdt_pool,
            tc.tile_pool(name="stg", bufs=2) as stg_pool,
            tc.tile_pool(name="prepx", bufs=PCH + 1) as prepx,
            tc.tile_pool(name="prepqd", bufs=2) as prepqd,
            tc.tile_pool(name="stat", bufs=3) as stat,
            tc.tile_pool(name="psum", bufs=6, space="PSUM") as psum_pool,
            tc.tile_pool(name="outp", bufs=2) as outp,
        ):
            # bias row (partition 0) + ones for the K=1 bias matmul / warmer
            bias_sb = const.tile([P, NL], BF16)
            nc.sync.dma_start(out=bias_sb[0:1, :], in_=b_in[None, :])
            ones_sb = const.tile([P, NBLK], BF16)
            nc.vector.memset(ones_sb[:], 1.0)

            def quant_dequant_rows(src, dst, row0):
                """fp8 quantize+dequantize one [P, K] row-tile src->dst (DRAM)."""
                xts = []
                amax = stat.tile([P, KO], F32, tag="amax")
                for c in range(PCH):
                    xt = prepx.tile([P, PREK], BF16, tag="pt_in")
                    nc.sync.dma_start(out=xt[:], in_=src[row0 : row0 + P, bass.ts(c, PREK)])
                    nc.vector.tensor_reduce(
                        out=amax[:, c * GC : (c + 1) * GC],
                        in_=xt.rearrange("p (g c) -> p g c", c=P),
                        axis=mybir.AxisListType.X,
                        op=mybir.AluOpType.max,
                        apply_absolute_value=True,
                    )
                    xts.append(xt)
                nc.vector.tensor_scalar_max(amax[:], amax[:], EPS)
                inv = stat.tile([P, KO], F32, tag="inv")
                nc.vector.reciprocal(out=inv[:], in_=amax[:])
                nc.vector.tensor_scalar_mul(inv[:], inv[:], FP8_SCALE)
                sc = stat.tile([P, KO], F32, tag="sc")
                nc.vector.tensor_scalar_mul(sc[:], amax[:], 1.0 / FP8_SCALE)
                for c in range(PCH):
                    xt = xts[c]
                    qt = prepqd.tile([P, PREK], FP8, tag="pt_q")
                    dt_ = prepqd.tile([P, PREK], BF16, tag="pt_d")
                    nc.vector.tensor_tensor(
                        out=qt.rearrange("p (g c) -> p g c", c=P),
                        in0=xt.rearrange("p (g c) -> p g c", c=P),
                        in1=_bcast(inv, c * GC, GC, P),
                        op=mybir.AluOpType.mult,
                    )
                    nc.gpsimd.tensor_tensor(
                        out=dt_.rearrange("p (g c) -> p g c", c=P),
                        in0=qt.rearrange("p (g c) -> p g c", c=P),
                        in1=_bcast(sc, c * GC, GC, P),
                        op=mybir.AluOpType.mult,
                    )
                    nc.sync.dma_start(out=dst[row0 : row0 + P, bass.ts(c, PREK)], in_=dt_[:])

            def x_block_transpose(blk):
                """xd_dram rows [blk*XBLK, +XBLK) -> xdT_dram [ko, p, mrange]."""
                m0 = blk * XBLK
                for ko in range(KO):
                    stg = stg_pool.tile([P, XBLK], BF16, tag="stg")
                    nc.sync.dma_start_transpose(
                        out=stg[:],
                        in_=xd_dram[m0 : m0 + XBLK, bass.ts(ko, P)],
                    )
                    nc.sync.dma_start(out=xdt_dram[ko, :, m0 : m0 + XBLK], in_=stg[:])

            n_xblk = M // XBLK

            # x block 0 preproc first, then all W preproc
            for t in range(XBLK // P):
                quant_dequant_rows(x_in, xd_dram, t * P)
            for t in range(NL // P):
                quant_dequant_rows(w_in, wd_dram, t * P)

            # wd -> wdT (SBUF resident, K on partitions); gated per WSPL rows
            wdT = wdt_pool.tile([P, KO, NL], BF16)
            for ws in range(WSPL):
                for ko in range(KO):
                    nc.sync.dma_start_transpose(
                        out=wdT[:, ko, bass.ts(ws, NW)],
                        in_=wd_dram[bass.ds(ws * NW, NW), bass.ts(ko, P)],
                    )

            x_block_transpose(0)

            # PE warmer: keeps the PE busy (and HAM un-throttled) while the
            # head preproc runs; the result is discarded.
            if N_WARM:
                dpsum = psum_pool.tile([P, NBLK], F32, tag="ps", name="warmps")
                for i in range(N_WARM):
                    nc.tensor.matmul(
                        dpsum[:], ones_sb[:, :P], ones_sb[:],
                        start=(i == 0), stop=(i == N_WARM - 1),
                    )
                wsink = const.tile([1, NBLK], F32)
                nc.scalar.copy(out=wsink[:], in_=dpsum[0:1, :])
                nc.sync.dma_start(out=warm_sink, in_=wsink[:])

            panels_per_blk = XBLK // MP
            for mp in range(M // MP):
                blk = mp // panels_per_blk
                if mp % panels_per_blk == 0:
                    # JIT: preproc + transpose the NEXT x block while this one runs
                    if blk + 1 < n_xblk:
                        for t in range(XBLK // P):
                            quant_dequant_rows(x_in, xd_dram, (blk + 1) * XBLK + t * P)
                        x_block_transpose(blk + 1)
                mrow0 = mp * MP
                xdT = xdt_pool.tile([P, KO, MP], BF16, tag="xdT")
                nc.sync.dma_start(
                    out=xdT[:],
                    in_=xdt_dram.rearrange("ko p m -> p ko m")[:, :, mrow0 : mrow0 + MP],
                )
                for ms in range(MPT):
                    ot = outp.tile([P, NL], BF16, tag="osb")
                    psums = [
                        psum_pool.tile([P, NBLK], F32, tag="ps", name=f"ps{i}")
                        for i in range(NB)
                    ]
                    for nbi in range(NB):
                        # bias row opens the accumulation group (K=1 matmul)
                        nc.tensor.matmul(
                            psums[nbi][:],
                            ones_sb[0:1, :P],
                            bias_sb[0:1, bass.ts(nbi, NBLK)],
                            start=True,
                            stop=False,
                        )
                    for ko in range(KO):
                        for nbi in range(NB):
                            nc.tensor.matmul(
                                psums[nbi][:],
                                xdT[:, ko, bass.ts(ms, P)],
                                wdT[:, ko, bass.ts(nbi, NBLK)],
                                start=False,
                                stop=(ko == KO - 1),
                            )
                    for nbi in range(NB):
                        nc.scalar.copy(out=ot[:, bass.ts(nbi, NBLK)], in_=psums[nbi][:])
                    nc.sync.dma_start(
                        out=out[mrow0 + ms * P : mrow0 + (ms + 1) * P, :],
                        in_=ot[:],
                    )

    nc.compile()
    return nc


_CACHE = {}


def _get_program(M, K, NL, **kw):
    key = (M, K, NL, tuple(sorted(kw.items())))
    if key not in _CACHE:
        _CACHE[key] = build_core_program(M, K, NL, **kw)
    return _CACHE[key]


def kernel(x, W, bias, chunk_size=128, int8=0, **_unused):
    """Full-input entry: shards across 8 NeuronCores (column-parallel) and
    returns the full [M, N] output."""
    from concourse.bass_utils import run_bass_kernel_spmd

    assert int(chunk_size) == 128 and int(int8) == 0
    x = np.asarray(x)
    W = np.asarray(W)
    bias = np.asarray(bias)
    M, K = x.shape
    N = W.shape[0]
    n_cores = 8
    assert N % n_cores == 0
    NL = N // n_cores

    nc = _get_program(M, K, NL)

    bf = ml_dtypes.bfloat16
    xb = np.ascontiguousarray(x.astype(bf, copy=False))
    in_maps = []
    for i in range(n_cores):
        in_maps.append(
            {
                "x": xb,
                "w": np.ascontiguousarray(W[i * NL : (i + 1) * NL].astype(bf, copy=False)),
                "bias": np.ascontiguousarray(bias[i * NL : (i + 1) * NL].astype(bf, copy=False)),
            }
        )

    res = run_bass_kernel_spmd(nc, in_maps, core_ids=list(range(n_cores)))
    outs = [res.results[i]["out"] for i in range(n_cores)]
    full = np.concatenate(outs, axis=1)
    return full.astype(x.dtype, copy=False)
